# revision 8
# baseline (speedup 1.0000x reference)
"""Trainium2 Bass kernel for InterventionAwareStructure loss.

loss = sum_b,i,d A[b,i,d] * mask[regimes[b], d] / count   (scalar)

Data-parallel over batch across 8 NeuronCores. Each core:
  - streams its A shard [32, 512, 512] from HBM in 1 MB chunks on the
    SP HWDGE ring (fp32 bits re-tagged as fp32r via a dram-tensor
    bitcast, so no SWDGE cast path and no Q7 descriptor-emission
    latency),
  - TensorE reduces each chunk over the source axis i with a one-hot
    stationary column (1 cycle/row in fp32r), accumulating ALL chunks
    into a single [32, 512] PSUM tile,
  - one final VectorE multiply with the gathered mask rows + free-axis
    reduce produces per-batch dots [32, 1],
  - the host sums the 8x32 partial dots and divides by the mask count.

The tiny gather mask[regimes] (256x512) and the final scalar reduction
are done on host; they are negligible next to the 256 MB stream of A.
"""

import numpy as np

import concourse.bass as bass
import concourse.tile as tile
from concourse import bacc, mybir
from concourse.bass_utils import run_bass_kernel_spmd

INTERVENTION_STRENGTH = 1.0

N_CORES = 8
B, N_REGIMES, D = 256, 16, 512
B_SH = B // N_CORES          # 32 batch items per core
ROWS_PER_PART = D // 128     # 4 source rows per partition per batch item
FREE = ROWS_PER_PART * D     # 2048 f32 per partition per 1 MB chunk
NMM = FREE // D              # 4 matmuls of free-dim 512 per chunk

_CACHED_NC = None


def _build_nc() -> bass.Bass:
    nc = bacc.Bacc()
    f32 = mybir.dt.float32
    f32r = mybir.dt.float32r

    # fp32 bits, tagged fp32r so HWDGE can move them without a cast.
    a = nc.dram_tensor("a", [B_SH, D, D], f32, kind="ExternalInput").bitcast(f32r)
    m = nc.dram_tensor("m", [B_SH, D], f32, kind="ExternalInput")
    out = nc.dram_tensor("out", [B_SH, 1], f32, kind="ExternalOutput")

    # One-hot stationaries: W[p, b*B_SH + q] = 1 iff q == b.  Matmul for
    # chunk b with stationary W[:, b*B_SH:(b+1)*B_SH] adds colsum(A_b)
    # into row b of the shared [32, 512] PSUM tile and zeros elsewhere.
    # (Aligned 32-column slices: the f32r weight-load path faults on
    # odd-offset stationary APs, so no sliding-window trick here.)
    w_np = np.zeros((128, B_SH * B_SH), dtype=np.float32)
    for b in range(B_SH):
        w_np[:, b * B_SH + b] = 1.0
    wsel = nc.inline_tensor(w_np, "wsel").bitcast(f32r)

    # Chunk b -> SBUF tile [128, FREE]: partition ih holds source rows
    # i = ih*NMM + il of batch item b; free axis = (il, d).
    a_view = a.rearrange("b (ih il) d -> b ih (il d)", ih=128)

    with tile.TileContext(nc) as tc:
        with (
            tc.tile_pool(name="big", bufs=16) as big_pool,
            tc.tile_pool(name="small", bufs=1) as small_pool,
            tc.tile_pool(name="psum", bufs=1, space="PSUM") as psum_pool,
        ):
            w_t = small_pool.tile([128, B_SH * B_SH], f32r)
            nc.scalar.dma_start(w_t[:], wsel[:])
            mask_t = small_pool.tile([B_SH, D], f32)
            nc.scalar.dma_start(mask_t[:], m[:, :])

            # Alternate chunks between the two HWDGE rings (SP + ACT):
            # each ring's per-DMA completion handling hides behind the
            # other ring's drain, keeping all 16 SDMA engines saturated.
            rings = [nc.sync, nc.scalar]
            a_tiles = []
            for b in range(B_SH):
                ring = rings[b % 2]
                a_t = big_pool.tile([128, FREE], f32r, tag="a")
                if b == B_SH - 1:
                    # Split the last chunk so the tail matmuls start on
                    # its first half while the second half streams.
                    half = FREE // 2
                    ring.dma_start(a_t[:, :half], a_view[b][:, :half])
                    rings[(b + 1) % 2].dma_start(a_t[:, half:], a_view[b][:, half:])
                else:
                    ring.dma_start(a_t[:], a_view[b])
                a_tiles.append(a_t)

            ps = psum_pool.tile([B_SH, D], f32)
            for b in range(B_SH):
                a_t = a_tiles[b]
                w_b = w_t[:, b * B_SH:(b + 1) * B_SH]
                for j in range(NMM):
                    nc.tensor.matmul(
                        ps[:],
                        w_b,
                        a_t[:, j * D:(j + 1) * D],
                        start=(b == 0 and j == 0),
                        stop=(b == B_SH - 1 and j == NMM - 1),
                    )

            tmp = small_pool.tile([B_SH, D], f32)
            nc.vector.tensor_mul(tmp[:], ps[:], mask_t[:])
            o_t = small_pool.tile([B_SH, 1], f32)
            nc.vector.reduce_sum(o_t[:], tmp[:], axis=mybir.AxisListType.X)
            nc.scalar.dma_start(out[:], o_t[:])

    nc.finalize()
    return nc


def _get_nc() -> bass.Bass:
    global _CACHED_NC
    if _CACHED_NC is None:
        _CACHED_NC = _build_nc()
    return _CACHED_NC


def _run(a_shards, m_shards, **run_kwargs):
    nc = _get_nc()
    in_maps = [
        {"a": np.ascontiguousarray(a_shards[c]), "m": np.ascontiguousarray(m_shards[c])}
        for c in range(N_CORES)
    ]
    return run_bass_kernel_spmd(nc, in_maps, list(range(N_CORES)), **run_kwargs)


def kernel(A_per_env, intervention_mask, regimes, _run_kwargs=None):
    A_per_env = np.asarray(A_per_env, dtype=np.float32)
    intervention_mask = np.asarray(intervention_mask, dtype=np.float32)
    regs = np.asarray(regimes).astype(np.int64)

    n_regimes = intervention_mask.shape[0]
    valid = regs < n_regimes
    e = np.clip(regs, 0, n_regimes - 1)
    masks = intervention_mask[e] * valid[:, None].astype(np.float32)  # [B, D]

    a_shards = [A_per_env[c * B_SH:(c + 1) * B_SH] for c in range(N_CORES)]
    m_shards = [masks[c * B_SH:(c + 1) * B_SH] for c in range(N_CORES)]

    res = _run(a_shards, m_shards, **(_run_kwargs or {}))
    num = np.float64(0.0)
    for c in range(N_CORES):
        num += res.results[c]["out"].astype(np.float64).sum()

    count = masks.astype(np.float64).sum()
    loss = num / count if count > 0 else num
    out = np.asarray(INTERVENTION_STRENGTH * loss, dtype=np.float32)
    if _run_kwargs is not None:
        return out, res
    return out
